# revision 61
# baseline (speedup 1.0000x reference)
"""Trainium2 Bass kernel for AssignmentWeightedAverage (nms_detection).

cost[m, n] = 0.4*(1 - box_iou) + 0.3*(1 - mask_iou) + 0.3*euclid(feat)

The heavy part is mask_iou's intersection matrix: a [256, 256] Gram matrix
over 256x(480*854) boolean masks (~105 MB each).  Strategy: shard the
CONTRACTION (pixel) axis across the 8 cores -- each core reads 1/8 of both
masks (~26 MB), computes a partial intersection Gram + partial areas, then
a fp16 AllReduce sums the partials; each core finishes the tiny box-iou /
reid / combine math for its 32-column stripe and the host concatenates.

Key tricks:
- masks stay RAW 0/1 bytes, declared fp8e4: 0x01 is the subnormal 2^-9, so
  matmul products are exactly 2^-18 and the f32 PSUM accumulation is exact;
  one 2^18 rescale during PSUM evacuation recovers exact counts.  No host
  value conversion, no DMA cast (1 byte/elem HBM + SBUF traffic).
- track-side mask areas ride along as a ones-column in the rhs (area1).
- current-side areas: the DVE taps the same SBUF bytes bitcast as u16 and
  integer-adds tile-blocks at 2x rate; byte sums never overflow; one
  and/sub extract + fp16 pack + a tiny ones-matmul gives area2.
- ALL mask bytes (and the feature matrices) stream on a single DMA queue:
  two concurrent queues measurably REDUCE total HBM throughput, and the
  per-core share (~340 B/ns) is the wall.  Small head chunks start the PE
  early; a tapered tail keeps the final PE lag ~1us.
- partition_id() loads and strided aux DMAs are kept off the critical
  queues (descriptor cost and engine-stream stalls dominate small DMAs).
- cross-core partials travel as one fp16 AllReduce (areas pre-scaled by
  1/8 so the 8-way sum stays under fp16 max) into a Shared-output buffer;
  per-partition 1028B records make staging and readback single-line DMAs;
  a tiny dummy AllReduce issued first absorbs the ~50us ncfw warmup
  behind the mask phase.
"""

import numpy as np
import ml_dtypes

from concourse import bass, bacc, mybir, tile
from concourse.bass_utils import run_bass_kernel_spmd

N1 = 256
N2 = 256
HW = 480 * 854            # 409920
D = 512
NCORES = 8
KPC = HW // NCORES        # 51240 pixels per core
TPC = (KPC + 127) // 128  # 401 K-tiles of 128 (last padded)
KP = TPC * 128            # 51328
M2T = 258                 # per-tile rhs width: 256 data + ones + 1 pad (even)
M2H = M2T // 2            # 136 u16 lanes per tile
AW = 8                    # accq accumulator width in K-tiles
# small head chunks (PE starts early), 24-tile body (fewer queue boundary
# turnarounds), tapered tail (short final PE lag)
SIZES = [4, 6, 8, 10, 12] + [24] * 14 + [20, 5]   # chunk tiles (sum=TPC)
MT = 256 + M2T            # bytes per tile in the merged chunk layout
NS = 32                   # output columns per core stripe
SH = 256 * NS + 256 + NS  # ReduceScatter shard: inter[256,32] + area1 + area2
W_BOX, W_MASK, W_REID = 0.4, 0.3, 0.3
RESCALE = float(2 ** 18)  # undo the fp8-subnormal 2^-18 product scale

f16 = mybir.dt.float16
f32 = mybir.dt.float32
bf16 = mybir.dt.bfloat16
u16 = mybir.dt.uint16
f8 = mybir.dt.float8e4
COPY = mybir.ActivationFunctionType.Copy
A = mybir.AluOpType
DR = mybir.MatmulPerfMode.DoubleRow

_CACHE = {}


def _build():
    if "nc" in _CACHE:
        return _CACHE["nc"]
    nc = bacc.Bacc("TRN2", target_bir_lowering=False, debug=False,
                   num_devices=NCORES)
    mdd = nc.dram_tensor("md", [128, TPC * MT], f8, kind="ExternalInput")
    tftd = nc.dram_tensor("tft", [D, N1], f32, kind="ExternalInput")
    cftd = nc.dram_tensor("cft", [D, N2], f32, kind="ExternalInput")
    tbd = nc.dram_tensor("tb", [N1, 4], f32, kind="ExternalInput")
    cbtd = nc.dram_tensor("cbt", [4, N2], f32, kind="ExternalInput")
    outd = nc.dram_tensor("out", [N1, N2], f32, kind="ExternalOutput")

    assert sum(SIZES) == TPC
    chunks = []
    s = 0
    for c in SIZES:
        chunks.append((s, c))
        s += c

    with tile.TileContext(nc) as tc:
        with tc.tile_pool(name="pm1", bufs=5) as pm1, \
             tc.tile_pool(name="pone", bufs=1) as pone, \
             tc.tile_pool(name="pmisc", bufs=1) as pmisc, \
             tc.tile_pool(name="pwork", bufs=2) as pwork, \
             tc.tile_pool(name="pps", bufs=1, space="PSUM") as pps, \
             tc.tile_pool(name="psc", bufs=3, space="PSUM") as psc, \
             tc.tile_pool(name="pdram", bufs=1, space="DRAM") as pdram:

            # dummy collective first: absorbs the first-trigger ncfw warmup
            # so the real ReduceScatter starts with ~1us delay instead of ~12
            dmy_in = pdram.tile([8], f32, tag="dmy_in")
            dmy_out = pdram.tile([8], f32, tag="dmy_out",
                                 addr_space="Shared")
            nc.gpsimd.collective_compute(
                "AllReduce", A.add,
                replica_groups=[[c] for c in range(NCORES)],
                ins=[dmy_in[:].opt()], outs=[dmy_out[:].opt()])

            accq = pmisc.tile([128, AW * M2H], u16, tag="accq")
            ones16_d = nc.inline_tensor(
                np.full((128, 1), 0.125, np.float16), name="ones16_d")
            ones16 = pone.tile([128, 1], f16, tag="ones16")
            onesb_d = nc.inline_tensor(np.ones((128, 1), ml_dtypes.bfloat16),
                                       name="onesb_d")
            onesb = pone.tile([128, 1], bf16, tag="onesb")
            # constants + boxes on the scalar ring, away from the mask queue
            nc.scalar.dma_start(ones16[:], ones16_d[:])
            nc.scalar.dma_start(onesb[:], onesb_d[:])
            # partition_id loads hit DRAM (~2-7us); issue the vector one now
            # (its queue has slack) and the sync one between early chunk
            # triggers so it never delays chunk 0 or stalls the stream
            rvv = nc.vector.partition_id()
            r32v = rvv * NS
            rvsc = nc.scalar.partition_id()
            r32sc = rvsc * NS
            tb_sb = pmisc.tile([128, 2, 4], f32, tag="tb_sb")
            nc.scalar.dma_start(tb_sb[:],
                                tbd[:].rearrange("(h p) c -> p h c", p=128))
            stage = pmisc.tile([1, 6 * 256], f32, tag="stage")
            for i in range(4):
                nc.scalar.dma_start(stage[0:1, i * 256:(i + 1) * 256],
                                    cbtd[i:i + 1, :])

            ps0 = pps.tile([128, 257], f32, tag="ps0")
            ps1 = pps.tile([128, 257], f32, tag="ps1")

            # ---- mask Gram loop (the heavy part) ----
            # one merged [m1-block | m2-block] DMA per chunk, single queue;
            # the feature loads ride the SAME queue (slot ~6) -- a second
            # concurrent queue measurably degrades total HBM throughput
            tf32 = pmisc.tile([128, 4, N1], f32, tag="tf32")
            cf32 = pmisc.tile([128, 4, N2], f32, tag="cf32")
            init = 0
            r32s = None
            for ci, (s0, cnt) in enumerate(chunks):
                td = pm1.tile([128, cnt * MT], f8, tag="td")
                nc.sync.dma_start(td[:], mdd[:, s0 * MT:(s0 + cnt) * MT])
                t1 = td[:, 0:cnt * 256]
                t2 = td[:, cnt * 256:cnt * MT]
                if ci == 4:
                    rvs = nc.sync.partition_id()
                    r32s = rvs * NS
                # fp8 DoubleRow: two K-tiles per matmul instruction
                t = 0
                while t < cnt:
                    g = s0 + t
                    if t + 1 < cnt:
                        l3 = t1[:, t * 256:(t + 2) * 256].rearrange(
                            "p (j m) -> p j m", j=2)
                        r3 = t2[:, t * M2T:(t + 2) * M2T].rearrange(
                            "p (j w) -> p j w", j=2)[:, :, 0:257]
                        nc.tensor.matmul(ps0[:], l3[:, :, 0:128], r3,
                                         perf_mode=DR,
                                         start=(g == 0), stop=(g + 2 == TPC))
                        nc.tensor.matmul(ps1[:], l3[:, :, 128:256], r3,
                                         perf_mode=DR,
                                         start=(g == 0), stop=(g + 2 == TPC))
                        t += 2
                    else:
                        lc = t * 256
                        rhs = t2[:, t * M2T:t * M2T + 257]
                        nc.tensor.matmul(ps0[:], t1[:, lc:lc + 128], rhs,
                                         start=(g == 0), stop=(g == TPC - 1))
                        nc.tensor.matmul(ps1[:], t1[:, lc + 128:lc + 256], rhs,
                                         start=(g == 0), stop=(g == TPC - 1))
                        t += 1
                # DVE tap for area2: u16 integer adds over the same bytes
                for off in range(0, cnt, AW):
                    w = min(AW, cnt - off)
                    a = min(w, init)
                    if a > 0:
                        nc.vector.tensor_add(
                            accq[:, :a * M2H], accq[:, :a * M2H],
                            t2[:, off * M2T:(off + a) * M2T].bitcast(u16))
                    if w > init:
                        nc.vector.tensor_copy(
                            accq[:, init * M2H:w * M2H],
                            t2[:, (off + init) * M2T:(off + w) * M2T].bitcast(
                                u16))
                        init = w
            # feature loads ride the tail of the mask queue: delaying them
            # past the last mask chunk shortens the PE-critical stream, and
            # their consumers (the reid Gram) only matter post-collective
            nc.sync.dma_start(
                tf32[:], tftd[:].rearrange("(p i) n -> p i n", p=128))
            nc.sync.dma_start(
                cf32[:], cftd[:].rearrange("(p i) n -> p i n", p=128))

            # fold accq's AW tile-blocks down to 2 (byte sums stay <= 255)
            assert init == AW
            cur = AW
            while cur > 2:
                if cur % 2:
                    nc.vector.tensor_add(
                        accq[:, :M2H], accq[:, :M2H],
                        accq[:, (cur - 1) * M2H:cur * M2H])
                    cur -= 1
                    if cur == 2:
                        break
                h = cur // 2
                nc.vector.tensor_add(accq[:, :h * M2H], accq[:, :h * M2H],
                                     accq[:, h * M2H:2 * h * M2H])
                cur = h
            # extract byte lanes: lo = even pixels' sums, hi = 256*odd sums
            lo2 = pmisc.tile([128, 2 * M2H], u16, tag="lo2")
            nc.vector.tensor_scalar(lo2[:], accq[:, :2 * M2H], 0x00FF, None,
                                    op0=A.bitwise_and)
            hi2 = pmisc.tile([128, 2 * M2H], u16, tag="hi2")
            nc.vector.tensor_sub(hi2[:], accq[:, :2 * M2H], lo2[:])
            af = pmisc.tile([128, 2 * M2T], f16, tag="af")
            # keep the whole area2 chain on the DVE: no cross-engine hops
            # and no contention with scalar's evac at mask-end
            nc.vector.tensor_scalar(af[:, 0:2 * M2H], lo2[:], 1.0, None,
                                    op0=A.mult)
            nc.vector.tensor_scalar(af[:, 2 * M2H:4 * M2H], hi2[:],
                                    1.0 / 256.0, None, op0=A.mult)
            nc.vector.tensor_add(af[:, 0:M2H], af[:, 0:M2H],
                                 af[:, M2H:2 * M2H])
            nc.vector.tensor_add(af[:, M2H:2 * M2H], af[:, 2 * M2H:3 * M2H],
                                 af[:, 3 * M2H:4 * M2H])
            # af[:, 0:130] = even-pixel counts, af[:, 130:260] = odd
            psA2 = psc.tile([1, 256], f32, tag="scratch")
            rhsA2 = af[:, 0:2 * M2H].rearrange("p (a b) -> p a b", a=2)[:, :, 0:128]
            # ones16 holds 1/8 so area2 partials arrive pre-scaled for fp16
            nc.tensor.matmul(psA2[:], ones16[:], rhsA2, start=True, stop=True)

            # ---- evacuate partials (rescaled 2^18) to fp16 ----
            # the full per-partition record [inter h0|h1 | area1 h0,h1]
            # assembles in ONE tile so staging is a single contiguous DMA;
            # area1 pre-scaled 1/8 so the 8-way fp16 sum stays < 65504
            # h0 evacuates on scalar, h1 on vector, in parallel
            cc_sb = pmisc.tile([128, 514], f16, tag="cc_sb")
            nc.scalar.activation(cc_sb[:, 0:256], ps0[:, 0:256], COPY,
                                 scale=RESCALE)
            nc.vector.tensor_scalar(cc_sb[:, 256:512], ps1[:, 0:256],
                                    RESCALE, None, op0=A.mult)
            nc.scalar.activation(cc_sb[:, 512:513], ps0[:, 256:257], COPY,
                                 scale=RESCALE / 8.0)
            nc.vector.tensor_scalar(cc_sb[:, 513:514], ps1[:, 256:257],
                                    RESCALE / 8.0, None, op0=A.mult)
            arow = pmisc.tile([1, 256], f16, tag="arow")
            # area2 packed [even|odd] -> natural order row
            nc.vector.tensor_copy(
                arow[:, 0:256],
                psA2[:].rearrange("p (s q) -> p q s", s=2))

            # ---- AllReduce (Shared-output fast path) of the partials ----
            # layout: per-partition records [p*514 + 0:512] = inter both
            # halves, [512:514] = area1[p,h]/8; then [65792:66048] area2/8.
            # All three staging DMAs are contiguous (no replication).
            CCN = 128 * 514 + 256
            cc_in = pdram.tile([CCN], f16, tag="cc_in")
            ar_out = pdram.tile([CCN], f16, tag="ar_out", addr_space="Shared")
            # stage in halves so the h0 (scalar-evacuated) part ships while
            # the vector engine is still finishing h1
            Xq = cc_in[0:128 * 514].rearrange("(p q) -> p q", q=514)
            nc.sync.dma_start(Xq[:, 0:256], cc_sb[:, 0:256])
            nc.sync.dma_start(Xq[:, 256:514], cc_sb[:, 256:514])
            nc.scalar.dma_start(
                cc_in[128 * 514:CCN].rearrange("(x j) -> x j", x=1),
                arow[:, 0:256])
            nc.gpsimd.collective_compute(
                "AllReduce", A.add,
                replica_groups=[list(range(NCORES))],
                ins=[cc_in[:].opt()], outs=[ar_out[:].opt()],
            )

            # ---- local stripe work that overlaps the collective ----
            # features were loaded f32 on the mask queue (contiguous
            # per-partition blocks); cast to bf16 on-chip.  Contraction
            # order over feature dims doesn't matter as long as both sides
            # use the same layout.
            tf_sb = pmisc.tile([128, 4, N1], bf16, tag="tf_sb")
            nc.scalar.activation(tf_sb[:], tf32[:], COPY, scale=1.0)
            cf_sb = pmisc.tile([128, 4, N2], bf16, tag="cf_sb")
            nc.scalar.activation(cf_sb[:], cf32[:], COPY, scale=1.0)
            # one bank holds both Gram halves; groups are serialized so the
            # second start=True only clears has_written bits of a DONE group
            psG = pps.tile([128, 2 * N2], f32, tag="psG")
            psG0 = psG[:, 0:N2]
            psG1 = psG[:, N2:2 * N2]
            for i in range(4):
                nc.tensor.matmul(psG0, tf_sb[:, i, 0:128], cf_sb[:, i, :],
                                 start=(i == 0), stop=(i == 3))
            for i in range(4):
                nc.tensor.matmul(psG1, tf_sb[:, i, 128:256], cf_sb[:, i, :],
                                 start=(i == 0), stop=(i == 3))
            sqt = pmisc.tile([128, 4, N1], bf16, tag="sqt")
            nc.scalar.square(sqt[:], tf_sb[:])
            sqc = pmisc.tile([128, 4, N2], bf16, tag="sqc")
            nc.scalar.square(sqc[:], cf_sb[:])
            psN1 = pps.tile([128, 2], f32, tag="psN1")
            psN1h0 = psN1[:, 0:1]
            psN1h1 = psN1[:, 1:2]
            psN2 = pps.tile([1, N2], f32, tag="psN2")
            for i in range(4):
                nc.tensor.matmul(psN1h0, sqt[:, i, 0:128], onesb[:],
                                 start=(i == 0), stop=(i == 3))
            for i in range(4):
                nc.tensor.matmul(psN1h1, sqt[:, i, 128:256], onesb[:],
                                 start=(i == 0), stop=(i == 3))
            for i in range(4):
                nc.tensor.matmul(psN2[:], onesb[:], sqc[:, i, :],
                                 start=(i == 0), stop=(i == 3))

            # boxes
            tmpc = pmisc.tile([1, 256], f32, tag="tmpc")
            nc.vector.tensor_sub(stage[:, 1024:1280], stage[:, 512:768],
                                 stage[:, 0:256])
            nc.vector.tensor_sub(tmpc[:], stage[:, 768:1024], stage[:, 256:512])
            nc.vector.tensor_mul(stage[:, 1024:1280], stage[:, 1024:1280],
                                 tmpc[:])
            nc.scalar.copy(stage[0:1, 1280:1536], psN2[:])
            bc = pmisc.tile([128, 6 * 256], f32, tag="bc")
            nc.gpsimd.partition_broadcast(bc[:], stage[0:1, :])

            def bcs(c):  # this core's n-stripe of broadcast row c
                return bc[:, bass.ds(r32v + c * 256, NS)]

            # box iou + reid for this core's stripe (no collective needed)
            pred2 = pwork.tile([128, 2, NS], f32, tag="pred2")
            fin2 = pwork.tile([128, 2, NS], f32, tag="fin2")
            for h in range(2):
                tbh = tb_sb[:, h, :]
                tx1, ty1 = tbh[:, 0:1], tbh[:, 1:2]
                tx2, ty2 = tbh[:, 2:3], tbh[:, 3:4]
                wx = pwork.tile([128, NS], f32, tag="wx")
                wy = pwork.tile([128, NS], f32, tag="wy")
                t0 = pwork.tile([128, NS], f32, tag="t0")
                nc.vector.tensor_scalar(wx[:], bcs(2), tx2, None, op0=A.min)
                nc.vector.tensor_scalar(t0[:], bcs(0), tx1, None, op0=A.max)
                nc.vector.tensor_sub(wx[:], wx[:], t0[:])
                nc.vector.tensor_scalar(wx[:], wx[:], 0.0, None, op0=A.max)
                nc.vector.tensor_scalar(wy[:], bcs(3), ty2, None, op0=A.min)
                nc.vector.tensor_scalar(t0[:], bcs(1), ty1, None, op0=A.max)
                nc.vector.tensor_sub(wy[:], wy[:], t0[:])
                nc.vector.tensor_scalar(wy[:], wy[:], 0.0, None, op0=A.max)
                ib = pwork.tile([128, NS], f32, tag="ib")
                nc.vector.tensor_mul(ib[:], wx[:], wy[:])
                td1 = pwork.tile([128, 1], f32, tag="td1")
                td2 = pwork.tile([128, 1], f32, tag="td2")
                nc.vector.tensor_scalar(td1[:], tx2, tx1, None, op0=A.subtract)
                nc.vector.tensor_scalar(td2[:], ty2, ty1, None, op0=A.subtract)
                nc.vector.tensor_mul(td1[:], td1[:], td2[:])
                ub = pwork.tile([128, NS], f32, tag="ub")
                nc.vector.scalar_tensor_tensor(ub[:], bcs(4), td1[:], ib[:],
                                               op0=A.add, op1=A.subtract)
                nc.vector.reciprocal(ub[:], ub[:])
                biou = pwork.tile([128, NS], f32, tag="biou")
                nc.vector.tensor_mul(biou[:], ib[:], ub[:])
                nc.vector.tensor_scalar(pred2[:, h, :], ib[:], 0.0, None,
                                        op0=A.is_gt)
                # reid euclid
                psN1h = psN1h0 if h == 0 else psN1h1
                sq = pwork.tile([128, NS], f32, tag="sq")
                nc.vector.scalar_tensor_tensor(
                    sq[:], psG[:, bass.ds(r32v + h * N2, NS)], -2.0, bcs(5),
                    op0=A.mult, op1=A.add)
                nc.vector.tensor_scalar(sq[:], sq[:], psN1h, 0.0,
                                        op0=A.add, op1=A.max)
                reid = pwork.tile([128, NS], f32, tag="reid")
                nc.scalar.sqrt(reid[:], sq[:])
                fin = fin2[:, h, :]
                nc.vector.tensor_scalar(fin, biou[:], -W_BOX, W_BOX + W_MASK,
                                        op0=A.mult, op1=A.add)
                nc.vector.scalar_tensor_tensor(fin, reid[:], W_REID, fin,
                                               op0=A.mult, op1=A.add)

            # ---- read back the summed partials and finish the stripe ----
            # one full-record DMA (contiguous 1028B line per partition);
            # stripe slicing happens on-chip with the vector engine's
            # partition-id register
            rallA = pmisc.tile([128, 514], f16, tag="rallA")
            nc.sync.dma_start(
                rallA[:], ar_out[0:128 * 514].rearrange("(p q) -> p q", q=514))
            a2row = pmisc.tile([1, 256], f16, tag="a2row")
            nc.scalar.dma_start(
                a2row[:], ar_out[128 * 514:CCN].rearrange("(x j) -> x j", x=1))
            # casts on the DVE: scalar's sqrt evicted the COPY act table and
            # a mid-readback ACT_TABLE_LOAD costs ~1.5us on the critical path
            a2rowf = pmisc.tile([1, 256], f32, tag="a2rowf")
            nc.vector.tensor_scalar(a2rowf[:], a2row[:], 8.0, None,
                                    op0=A.mult)
            bc256 = pmisc.tile([128, 256], f32, tag="bc256")
            nc.gpsimd.partition_broadcast(bc256[:], a2rowf[:])
            a1f = pmisc.tile([128, 2], f32, tag="a1f")
            nc.vector.tensor_scalar(a1f[:], rallA[:, 512:514], 8.0, None,
                                    op0=A.mult)

            # um = (area2[n] + area1[m]) - inter, fused per half; the inter
            # stripe multiplies straight out of the f16 readback records
            interp = pwork.tile([128, 2, NS], f32, tag="interp")
            nc.vector.tensor_mul(
                interp[:],
                rallA[:, 0:512].rearrange("p (h n) -> p h n", h=2)[
                    :, :, bass.ds(r32v, NS)],
                pred2[:])
            um = pwork.tile([128, 2, NS], f32, tag="um")
            for h in range(2):
                nc.vector.scalar_tensor_tensor(
                    um[:, h, :], bc256[:, bass.ds(r32v, NS)],
                    a1f[:, h:h + 1], interp[:, h, :],
                    op0=A.add, op1=A.subtract)
            nc.vector.reciprocal(um[:], um[:])
            nc.vector.tensor_mul(interp[:], interp[:], um[:])
            nc.vector.scalar_tensor_tensor(fin2[:], interp[:], -W_MASK, fin2[:],
                                           op0=A.mult, op1=A.add)
            # output halves on separate queues
            odst = outd[:].rearrange("(h p) n -> p h n", h=2)
            nc.sync.dma_start(odst[:, 0, :][:, bass.ds(r32s, NS)],
                              fin2[:, 0, :])
            nc.scalar.dma_start(odst[:, 1, :][:, bass.ds(r32sc, NS)],
                                fin2[:, 1, :])

    nc.compile()
    _CACHE["nc"] = nc
    return nc


def _prep_mask_t(mask_u8, ones_col):
    """[256, HW] uint8 -> [8, 128, TPC, w] per-core transposed tile layout."""
    w = M2T if ones_col else 256
    out = np.zeros((NCORES, 128, TPC, w), dtype=np.uint8)
    if ones_col:
        out[..., 256] = 1
    for c in range(NCORES):
        chunk = mask_u8[:, c * KPC:(c + 1) * KPC]          # [256, 51240]
        ct = np.zeros((KP, N1), dtype=np.uint8)
        ct[:KPC] = chunk.T                                  # [51328, 256]
        ct = ct.reshape(TPC, 128, N1).transpose(1, 0, 2)    # [128, TPC, 256]
        out[c, :, :, :256] = ct
    return out


def kernel(track_features, current_features, track_boxes, current_boxes,
           track_time, current_time, track_masks, current_masks):
    tm = np.asarray(track_masks).reshape(N1, HW).astype(np.uint8, copy=False)
    cm = np.asarray(current_masks).reshape(N2, HW).astype(np.uint8, copy=False)
    m1 = _prep_mask_t(tm, ones_col=False)       # [8, 128, TPC, 256]
    m2 = _prep_mask_t(cm, ones_col=True)        # [8, 128, TPC, M2T]
    # merge chunk-wise: per chunk block = [m1 tiles | m2 tiles]
    md = np.empty((NCORES, 128, TPC * MT), dtype=np.uint8)
    s = 0
    off = 0
    for cnt in SIZES:
        w1 = cnt * 256
        w2 = cnt * M2T
        md[:, :, off:off + w1] = m1[:, :, s:s + cnt].reshape(NCORES, 128, w1)
        md[:, :, off + w1:off + w1 + w2] = m2[:, :, s:s + cnt].reshape(
            NCORES, 128, w2)
        s += cnt
        off += w1 + w2
    md = md.view(ml_dtypes.float8_e4m3)

    tft = np.ascontiguousarray(np.asarray(track_features, dtype=np.float32).T)
    cft = np.ascontiguousarray(np.asarray(current_features, dtype=np.float32).T)
    tb = np.ascontiguousarray(np.asarray(track_boxes, dtype=np.float32))
    cbt = np.ascontiguousarray(np.asarray(current_boxes, dtype=np.float32).T)

    in_maps = [
        {"md": md[c], "tft": tft, "cft": cft, "tb": tb, "cbt": cbt}
        for c in range(NCORES)
    ]
    nc = _build()
    res = run_bass_kernel_spmd(nc, in_maps, core_ids=list(range(NCORES)),
                               trace=_CACHE.get("trace", False))
    _CACHE["last_exec_time_ns"] = res.exec_time_ns
    out = np.empty((N1, N2), dtype=np.float32)
    for c in range(NCORES):
        out[:, c * NS:(c + 1) * NS] = np.asarray(
            res.results[c]["out"])[:, c * NS:(c + 1) * NS]
    return out


# revision 65
# speedup vs baseline: 1.0623x; 1.0623x over previous
"""Trainium2 Bass kernel for AssignmentWeightedAverage (nms_detection).

cost[m, n] = 0.4*(1 - box_iou) + 0.3*(1 - mask_iou) + 0.3*euclid(feat)

The heavy part is mask_iou's intersection matrix: a [256, 256] Gram matrix
over 256x(480*854) boolean masks (~105 MB each).  Strategy: shard the
CONTRACTION (pixel) axis across the 8 cores -- each core reads 1/8 of both
masks (~26 MB), computes a partial intersection Gram + partial areas, then
a fp16 AllReduce sums the partials; each core finishes the tiny box-iou /
reid / combine math for its 32-column stripe and the host concatenates.

Key tricks:
- masks stay RAW 0/1 bytes, declared fp8e4: 0x01 is the subnormal 2^-9, so
  matmul products are exactly 2^-18 and the f32 PSUM accumulation is exact;
  one 2^18 rescale during PSUM evacuation recovers exact counts.  No host
  value conversion, no DMA cast (1 byte/elem HBM + SBUF traffic).
- track-side mask areas ride along as a ones-column in the rhs (area1).
- current-side areas: the DVE taps the same SBUF bytes bitcast as u16 and
  integer-adds tile-blocks at 2x rate; byte sums never overflow; one
  and/sub extract + fp16 pack + a tiny ones-matmul gives area2.
- ALL mask bytes (and the feature matrices) stream on a single DMA queue:
  two concurrent queues measurably REDUCE total HBM throughput, and the
  per-core share (~340 B/ns) is the wall.  Small head chunks start the PE
  early; a tapered tail keeps the final PE lag ~1us.
- partition_id() loads and strided aux DMAs are kept off the critical
  queues (descriptor cost and engine-stream stalls dominate small DMAs).
- cross-core partials travel as one fp16 AllReduce (areas pre-scaled by
  1/8 so the 8-way sum stays under fp16 max) into a Shared-output buffer;
  per-partition 1028B records make staging and readback single-line DMAs;
  a tiny dummy AllReduce issued first absorbs the ~50us ncfw warmup
  behind the mask phase.
"""

import numpy as np
import ml_dtypes

from concourse import bass, bacc, mybir, tile
from concourse.bass_utils import run_bass_kernel_spmd

N1 = 256
N2 = 256
HW = 480 * 854            # 409920
D = 512
NCORES = 8
KPC = HW // NCORES        # 51240 pixels per core
TPC = (KPC + 127) // 128  # 401 K-tiles of 128 (last padded)
KP = TPC * 128            # 51328
M2T = 258                 # per-tile rhs width: 256 data + ones + 1 pad (even)
M2H = M2T // 2            # 136 u16 lanes per tile
AW = 8                    # accq accumulator width in K-tiles
# small head chunks (PE starts early), 24-tile body (fewer queue boundary
# turnarounds), tapered tail (short final PE lag)
SIZES = [4, 6, 8, 10, 12] + [24] * 14 + [20, 5]   # chunk tiles (sum=TPC)
MT = 256 + M2T            # bytes per tile in the merged chunk layout
NS = 32                   # output columns per core stripe
SH = 256 * NS + 256 + NS  # ReduceScatter shard: inter[256,32] + area1 + area2
W_BOX, W_MASK, W_REID = 0.4, 0.3, 0.3
RESCALE = float(2 ** 18)  # undo the fp8-subnormal 2^-18 product scale

f16 = mybir.dt.float16
f32 = mybir.dt.float32
bf16 = mybir.dt.bfloat16
u16 = mybir.dt.uint16
f8 = mybir.dt.float8e4
COPY = mybir.ActivationFunctionType.Copy
A = mybir.AluOpType
DR = mybir.MatmulPerfMode.DoubleRow

_CACHE = {}


def _build():
    if "nc" in _CACHE:
        return _CACHE["nc"]
    nc = bacc.Bacc("TRN2", target_bir_lowering=False, debug=False,
                   num_devices=NCORES)
    mdd = nc.dram_tensor("md", [128, TPC * MT], f8, kind="ExternalInput")
    tftd = nc.dram_tensor("tft", [D, N1], f32, kind="ExternalInput")
    cftd = nc.dram_tensor("cft", [D, N2], f32, kind="ExternalInput")
    tbd = nc.dram_tensor("tb", [N1, 4], f32, kind="ExternalInput")
    cbtd = nc.dram_tensor("cbt", [4, N2], f32, kind="ExternalInput")
    outd = nc.dram_tensor("out", [N1, N2], f32, kind="ExternalOutput")

    assert sum(SIZES) == TPC
    chunks = []
    s = 0
    for c in SIZES:
        chunks.append((s, c))
        s += c

    with tile.TileContext(nc) as tc:
        with tc.tile_pool(name="pm1", bufs=5) as pm1, \
             tc.tile_pool(name="pone", bufs=1) as pone, \
             tc.tile_pool(name="pmisc", bufs=1) as pmisc, \
             tc.tile_pool(name="pwork", bufs=2) as pwork, \
             tc.tile_pool(name="pps", bufs=1, space="PSUM") as pps, \
             tc.tile_pool(name="psc", bufs=3, space="PSUM") as psc, \
             tc.tile_pool(name="pdram", bufs=1, space="DRAM") as pdram:

            # dummy collective first: absorbs the first-trigger ncfw warmup
            # so the real ReduceScatter starts with ~1us delay instead of ~12
            dmy_in = pdram.tile([8], f32, tag="dmy_in")
            dmy_out = pdram.tile([8], f32, tag="dmy_out",
                                 addr_space="Shared")
            nc.gpsimd.collective_compute(
                "AllReduce", A.add,
                replica_groups=[[c] for c in range(NCORES)],
                ins=[dmy_in[:].opt()], outs=[dmy_out[:].opt()])

            accq = pmisc.tile([128, AW * M2H], u16, tag="accq")
            ones16_d = nc.inline_tensor(
                np.full((128, 1), 0.125, np.float16), name="ones16_d")
            ones16 = pone.tile([128, 1], f16, tag="ones16")
            onesb_d = nc.inline_tensor(np.ones((128, 1), ml_dtypes.bfloat16),
                                       name="onesb_d")
            onesb = pone.tile([128, 1], bf16, tag="onesb")
            # constants + boxes on the scalar ring, away from the mask queue
            nc.scalar.dma_start(ones16[:], ones16_d[:])
            nc.scalar.dma_start(onesb[:], onesb_d[:])
            # partition_id loads hit DRAM (~2-7us); issue the vector one now
            # (its queue has slack) and the sync one between early chunk
            # triggers so it never delays chunk 0 or stalls the stream
            rvv = nc.vector.partition_id()
            r32v = rvv * NS
            rvsc = nc.scalar.partition_id()
            r32sc = rvsc * NS
            tb_sb = pmisc.tile([128, 2, 4], f32, tag="tb_sb")
            nc.scalar.dma_start(tb_sb[:],
                                tbd[:].rearrange("(h p) c -> p h c", p=128))
            stage = pmisc.tile([1, 6 * 256], f32, tag="stage")
            for i in range(4):
                nc.scalar.dma_start(stage[0:1, i * 256:(i + 1) * 256],
                                    cbtd[i:i + 1, :])

            ps0 = pps.tile([128, 257], f32, tag="ps0")
            ps1 = pps.tile([128, 257], f32, tag="ps1")

            # ---- mask Gram loop (the heavy part) ----
            # one merged [m1-block | m2-block] DMA per chunk, single queue;
            # the feature loads ride the SAME queue (slot ~6) -- a second
            # concurrent queue measurably degrades total HBM throughput
            tf32 = pmisc.tile([128, 4, N1], f32, tag="tf32")
            cf32 = pmisc.tile([128, 4, N2], f32, tag="cf32")
            init = 0
            r32s = None
            for ci, (s0, cnt) in enumerate(chunks):
                td = pm1.tile([128, cnt * MT], f8, tag="td")
                nc.sync.dma_start(td[:], mdd[:, s0 * MT:(s0 + cnt) * MT])
                t1 = td[:, 0:cnt * 256]
                t2 = td[:, cnt * 256:cnt * MT]
                if ci == 4:
                    rvs = nc.sync.partition_id()
                    r32s = rvs * NS
                # fp8 DoubleRow: two K-tiles per matmul instruction
                t = 0
                while t < cnt:
                    g = s0 + t
                    if t + 1 < cnt:
                        l3 = t1[:, t * 256:(t + 2) * 256].rearrange(
                            "p (j m) -> p j m", j=2)
                        r3 = t2[:, t * M2T:(t + 2) * M2T].rearrange(
                            "p (j w) -> p j w", j=2)[:, :, 0:257]
                        nc.tensor.matmul(ps0[:], l3[:, :, 0:128], r3,
                                         perf_mode=DR,
                                         start=(g == 0), stop=(g + 2 == TPC))
                        nc.tensor.matmul(ps1[:], l3[:, :, 128:256], r3,
                                         perf_mode=DR,
                                         start=(g == 0), stop=(g + 2 == TPC))
                        t += 2
                    else:
                        lc = t * 256
                        rhs = t2[:, t * M2T:t * M2T + 257]
                        nc.tensor.matmul(ps0[:], t1[:, lc:lc + 128], rhs,
                                         start=(g == 0), stop=(g == TPC - 1))
                        nc.tensor.matmul(ps1[:], t1[:, lc + 128:lc + 256], rhs,
                                         start=(g == 0), stop=(g == TPC - 1))
                        t += 1
                # DVE tap for area2: u16 integer adds over the same bytes
                for off in range(0, cnt, AW):
                    w = min(AW, cnt - off)
                    a = min(w, init)
                    if a > 0:
                        nc.vector.tensor_add(
                            accq[:, :a * M2H], accq[:, :a * M2H],
                            t2[:, off * M2T:(off + a) * M2T].bitcast(u16))
                    if w > init:
                        nc.vector.tensor_copy(
                            accq[:, init * M2H:w * M2H],
                            t2[:, (off + init) * M2T:(off + w) * M2T].bitcast(
                                u16))
                        init = w
            # feature loads ride the tail of the mask queue: delaying them
            # past the last mask chunk shortens the PE-critical stream, and
            # their consumers (the reid Gram) only matter post-collective
            nc.sync.dma_start(
                tf32[:], tftd[:].rearrange("(p i) n -> p i n", p=128))
            nc.sync.dma_start(
                cf32[:], cftd[:].rearrange("(p i) n -> p i n", p=128))

            # fold accq's AW tile-blocks down to 2 (byte sums stay <= 255)
            assert init == AW
            cur = AW
            while cur > 2:
                if cur % 2:
                    nc.vector.tensor_add(
                        accq[:, :M2H], accq[:, :M2H],
                        accq[:, (cur - 1) * M2H:cur * M2H])
                    cur -= 1
                    if cur == 2:
                        break
                h = cur // 2
                nc.vector.tensor_add(accq[:, :h * M2H], accq[:, :h * M2H],
                                     accq[:, h * M2H:2 * h * M2H])
                cur = h
            # extract byte lanes: lo = even pixels' sums, hi = 256*odd sums
            lo2 = pmisc.tile([128, 2 * M2H], u16, tag="lo2")
            nc.vector.tensor_scalar(lo2[:], accq[:, :2 * M2H], 0x00FF, None,
                                    op0=A.bitwise_and)
            hi2 = pmisc.tile([128, 2 * M2H], u16, tag="hi2")
            nc.vector.tensor_sub(hi2[:], accq[:, :2 * M2H], lo2[:])
            af = pmisc.tile([128, 2 * M2T], f16, tag="af")
            # keep the whole area2 chain on the DVE: no cross-engine hops
            # and no contention with scalar's evac at mask-end
            nc.vector.tensor_scalar(af[:, 0:2 * M2H], lo2[:], 1.0, None,
                                    op0=A.mult)
            nc.vector.tensor_scalar(af[:, 2 * M2H:4 * M2H], hi2[:],
                                    1.0 / 256.0, None, op0=A.mult)
            nc.vector.tensor_add(af[:, 0:M2H], af[:, 0:M2H],
                                 af[:, M2H:2 * M2H])
            nc.vector.tensor_add(af[:, M2H:2 * M2H], af[:, 2 * M2H:3 * M2H],
                                 af[:, 3 * M2H:4 * M2H])
            # af[:, 0:130] = even-pixel counts, af[:, 130:260] = odd
            psA2 = psc.tile([1, 256], f32, tag="scratch")
            rhsA2 = af[:, 0:2 * M2H].rearrange("p (a b) -> p a b", a=2)[:, :, 0:128]
            # ones16 holds 1/8 so area2 partials arrive pre-scaled for fp16
            nc.tensor.matmul(psA2[:], ones16[:], rhsA2, start=True, stop=True)

            # ---- evacuate partials (rescaled 2^18) to fp16 ----
            # the full per-partition record [inter h0|h1 | area1 h0,h1]
            # assembles in ONE tile so staging is a single contiguous DMA;
            # area1 pre-scaled 1/8 so the 8-way fp16 sum stays < 65504
            # h0 evacuates on scalar, h1 on vector, in parallel
            cc_sb = pmisc.tile([128, 514], f16, tag="cc_sb")
            nc.scalar.activation(cc_sb[:, 0:256], ps0[:, 0:256], COPY,
                                 scale=RESCALE)
            nc.vector.tensor_scalar(cc_sb[:, 256:512], ps1[:, 0:256],
                                    RESCALE, None, op0=A.mult)
            nc.scalar.activation(cc_sb[:, 512:513], ps0[:, 256:257], COPY,
                                 scale=RESCALE / 8.0)
            nc.vector.tensor_scalar(cc_sb[:, 513:514], ps1[:, 256:257],
                                    RESCALE / 8.0, None, op0=A.mult)
            arow = pmisc.tile([1, 256], f16, tag="arow")
            # area2 packed [even|odd] -> natural order row
            nc.vector.tensor_copy(
                arow[:, 0:256],
                psA2[:].rearrange("p (s q) -> p q s", s=2))

            # ---- AllReduce (Shared-output fast path) of the partials ----
            # layout: per-partition records [p*514 + 0:512] = inter both
            # halves, [512:514] = area1[p,h]/8; then [65792:66048] area2/8.
            # All three staging DMAs are contiguous (no replication).
            CCN = 128 * 514 + 256
            cc_in = pdram.tile([CCN], f16, tag="cc_in")
            ar_out = pdram.tile([CCN], f16, tag="ar_out", addr_space="Shared")
            # stage in halves so the h0 (scalar-evacuated) part ships while
            # the vector engine is still finishing h1
            Xq = cc_in[0:128 * 514].rearrange("(p q) -> p q", q=514)
            nc.sync.dma_start(Xq[:, 0:256], cc_sb[:, 0:256])
            nc.sync.dma_start(Xq[:, 256:514], cc_sb[:, 256:514])
            nc.scalar.dma_start(
                cc_in[128 * 514:CCN].rearrange("(x j) -> x j", x=1),
                arow[:, 0:256])
            nc.gpsimd.collective_compute(
                "AllReduce", A.add,
                replica_groups=[list(range(NCORES))],
                ins=[cc_in[:].opt()], outs=[ar_out[:].opt()],
            )

            # ---- local stripe work that overlaps the collective ----
            # features were loaded f32 on the mask queue (contiguous
            # per-partition blocks); cast to bf16 on-chip.  Contraction
            # order over feature dims doesn't matter as long as both sides
            # use the same layout.
            tf_sb = pmisc.tile([128, 4, N1], bf16, tag="tf_sb")
            nc.scalar.activation(tf_sb[:], tf32[:], COPY, scale=1.0)
            cf_sb = pmisc.tile([128, 4, N2], bf16, tag="cf_sb")
            nc.scalar.activation(cf_sb[:], cf32[:], COPY, scale=1.0)
            # one bank holds both Gram halves; groups are serialized so the
            # second start=True only clears has_written bits of a DONE group
            psG = pps.tile([128, 2 * N2], f32, tag="psG")
            psG0 = psG[:, 0:N2]
            psG1 = psG[:, N2:2 * N2]
            for i in range(4):
                nc.tensor.matmul(psG0, tf_sb[:, i, 0:128], cf_sb[:, i, :],
                                 start=(i == 0), stop=(i == 3))
            for i in range(4):
                nc.tensor.matmul(psG1, tf_sb[:, i, 128:256], cf_sb[:, i, :],
                                 start=(i == 0), stop=(i == 3))
            sqt = pmisc.tile([128, 4, N1], bf16, tag="sqt")
            nc.scalar.square(sqt[:], tf_sb[:])
            sqc = pmisc.tile([128, 4, N2], bf16, tag="sqc")
            nc.scalar.square(sqc[:], cf_sb[:])
            psN1 = pps.tile([128, 2], f32, tag="psN1")
            psN1h0 = psN1[:, 0:1]
            psN1h1 = psN1[:, 1:2]
            psN2 = pps.tile([1, N2], f32, tag="psN2")
            for i in range(4):
                nc.tensor.matmul(psN1h0, sqt[:, i, 0:128], onesb[:],
                                 start=(i == 0), stop=(i == 3))
            for i in range(4):
                nc.tensor.matmul(psN1h1, sqt[:, i, 128:256], onesb[:],
                                 start=(i == 0), stop=(i == 3))
            for i in range(4):
                nc.tensor.matmul(psN2[:], onesb[:], sqc[:, i, :],
                                 start=(i == 0), stop=(i == 3))

            # boxes
            tmpc = pmisc.tile([1, 256], f32, tag="tmpc")
            nc.vector.tensor_sub(stage[:, 1024:1280], stage[:, 512:768],
                                 stage[:, 0:256])
            nc.vector.tensor_sub(tmpc[:], stage[:, 768:1024], stage[:, 256:512])
            nc.vector.tensor_mul(stage[:, 1024:1280], stage[:, 1024:1280],
                                 tmpc[:])
            nc.scalar.copy(stage[0:1, 1280:1536], psN2[:])
            bc = pmisc.tile([128, 6 * 256], f32, tag="bc")
            nc.gpsimd.partition_broadcast(bc[:], stage[0:1, :])

            def bcs(c):  # this core's n-stripe of broadcast row c
                return bc[:, bass.ds(r32v + c * 256, NS)]

            # box iou + reid for this core's stripe (no collective needed)
            pred2 = pwork.tile([128, 2, NS], f32, tag="pred2")
            fin2 = pwork.tile([128, 2, NS], f32, tag="fin2")
            for h in range(2):
                tbh = tb_sb[:, h, :]
                tx1, ty1 = tbh[:, 0:1], tbh[:, 1:2]
                tx2, ty2 = tbh[:, 2:3], tbh[:, 3:4]
                wx = pwork.tile([128, NS], f32, tag="wx")
                wy = pwork.tile([128, NS], f32, tag="wy")
                t0 = pwork.tile([128, NS], f32, tag="t0")
                nc.vector.tensor_scalar(wx[:], bcs(2), tx2, None, op0=A.min)
                nc.vector.tensor_scalar(t0[:], bcs(0), tx1, None, op0=A.max)
                nc.vector.tensor_sub(wx[:], wx[:], t0[:])
                nc.vector.tensor_scalar(wx[:], wx[:], 0.0, None, op0=A.max)
                nc.vector.tensor_scalar(wy[:], bcs(3), ty2, None, op0=A.min)
                nc.vector.tensor_scalar(t0[:], bcs(1), ty1, None, op0=A.max)
                nc.vector.tensor_sub(wy[:], wy[:], t0[:])
                nc.vector.tensor_scalar(wy[:], wy[:], 0.0, None, op0=A.max)
                ib = pwork.tile([128, NS], f32, tag="ib")
                nc.vector.tensor_mul(ib[:], wx[:], wy[:])
                td1 = pwork.tile([128, 1], f32, tag="td1")
                td2 = pwork.tile([128, 1], f32, tag="td2")
                nc.vector.tensor_scalar(td1[:], tx2, tx1, None, op0=A.subtract)
                nc.vector.tensor_scalar(td2[:], ty2, ty1, None, op0=A.subtract)
                nc.vector.tensor_mul(td1[:], td1[:], td2[:])
                ub = pwork.tile([128, NS], f32, tag="ub")
                nc.vector.scalar_tensor_tensor(ub[:], bcs(4), td1[:], ib[:],
                                               op0=A.add, op1=A.subtract)
                nc.vector.reciprocal(ub[:], ub[:])
                biou = pwork.tile([128, NS], f32, tag="biou")
                nc.vector.tensor_mul(biou[:], ib[:], ub[:])
                nc.vector.tensor_scalar(pred2[:, h, :], ib[:], 0.0, None,
                                        op0=A.is_gt)
                # reid euclid
                psN1h = psN1h0 if h == 0 else psN1h1
                sq = pwork.tile([128, NS], f32, tag="sq")
                nc.vector.scalar_tensor_tensor(
                    sq[:], psG[:, bass.ds(r32v + h * N2, NS)], -2.0, bcs(5),
                    op0=A.mult, op1=A.add)
                nc.vector.tensor_scalar(sq[:], sq[:], psN1h, 0.0,
                                        op0=A.add, op1=A.max)
                reid = pwork.tile([128, NS], f32, tag="reid")
                nc.scalar.sqrt(reid[:], sq[:])
                fin = fin2[:, h, :]
                nc.vector.tensor_scalar(fin, biou[:], -W_BOX, W_BOX + W_MASK,
                                        op0=A.mult, op1=A.add)
                nc.vector.scalar_tensor_tensor(fin, reid[:], W_REID, fin,
                                               op0=A.mult, op1=A.add)

            # ---- read back the summed partials and finish the stripe ----
            # one full-record DMA (contiguous 1028B line per partition);
            # stripe slicing happens on-chip with the vector engine's
            # partition-id register
            rallA = pmisc.tile([128, 514], f16, tag="rallA")
            nc.sync.dma_start(
                rallA[:], ar_out[0:128 * 514].rearrange("(p q) -> p q", q=514))
            a2row = pmisc.tile([1, 256], f16, tag="a2row")
            nc.scalar.dma_start(
                a2row[:], ar_out[128 * 514:CCN].rearrange("(x j) -> x j", x=1))
            # casts on the DVE: scalar's sqrt evicted the COPY act table and
            # a mid-readback ACT_TABLE_LOAD costs ~1.5us on the critical path
            a2rowf = pmisc.tile([1, 256], f32, tag="a2rowf")
            nc.vector.tensor_scalar(a2rowf[:], a2row[:], 8.0, None,
                                    op0=A.mult)
            bc256 = pmisc.tile([128, 256], f32, tag="bc256")
            nc.gpsimd.partition_broadcast(bc256[:], a2rowf[:])
            a1f = pmisc.tile([128, 2], f32, tag="a1f")
            nc.vector.tensor_scalar(a1f[:], rallA[:, 512:514], 8.0, None,
                                    op0=A.mult)

            # um = (area2[n] + area1[m]) - inter, fused per half; the inter
            # stripe multiplies straight out of the f16 readback records
            interp = pwork.tile([128, 2, NS], f32, tag="interp")
            nc.vector.tensor_mul(
                interp[:],
                rallA[:, 0:512].rearrange("p (h n) -> p h n", h=2)[
                    :, :, bass.ds(r32v, NS)],
                pred2[:])
            um = pwork.tile([128, 2, NS], f32, tag="um")
            for h in range(2):
                nc.vector.scalar_tensor_tensor(
                    um[:, h, :], bc256[:, bass.ds(r32v, NS)],
                    a1f[:, h:h + 1], interp[:, h, :],
                    op0=A.add, op1=A.subtract)
            nc.vector.reciprocal(um[:], um[:])
            nc.vector.tensor_mul(interp[:], interp[:], um[:])
            nc.vector.scalar_tensor_tensor(fin2[:], interp[:], -W_MASK, fin2[:],
                                           op0=A.mult, op1=A.add)
            # output halves on separate queues
            odst = outd[:].rearrange("(h p) n -> p h n", h=2)
            nc.sync.dma_start(odst[:, 0, :][:, bass.ds(r32s, NS)],
                              fin2[:, 0, :])
            nc.scalar.dma_start(odst[:, 1, :][:, bass.ds(r32sc, NS)],
                                fin2[:, 1, :])

    nc.compile()
    _CACHE["nc"] = nc
    return nc


def _prep_mask_t(mask_u8, ones_col):
    """[256, HW] uint8 -> [8, 128, TPC, w] per-core transposed tile layout."""
    w = M2T if ones_col else 256
    out = np.zeros((NCORES, 128, TPC, w), dtype=np.uint8)
    if ones_col:
        out[..., 256] = 1
    for c in range(NCORES):
        chunk = mask_u8[:, c * KPC:(c + 1) * KPC]          # [256, 51240]
        ct = np.zeros((KP, N1), dtype=np.uint8)
        ct[:KPC] = chunk.T                                  # [51328, 256]
        ct = ct.reshape(TPC, 128, N1).transpose(1, 0, 2)    # [128, TPC, 256]
        out[c, :, :, :256] = ct
    return out


def kernel(track_features, current_features, track_boxes, current_boxes,
           track_time, current_time, track_masks, current_masks):
    tm = np.asarray(track_masks).reshape(N1, HW).astype(np.uint8, copy=False)
    cm = np.asarray(current_masks).reshape(N2, HW).astype(np.uint8, copy=False)
    m1 = _prep_mask_t(tm, ones_col=False)       # [8, 128, TPC, 256]
    m2 = _prep_mask_t(cm, ones_col=True)        # [8, 128, TPC, M2T]
    # merge chunk-wise: per chunk block = [m1 tiles | m2 tiles]
    md = np.empty((NCORES, 128, TPC * MT), dtype=np.uint8)
    s = 0
    off = 0
    for cnt in SIZES:
        w1 = cnt * 256
        w2 = cnt * M2T
        md[:, :, off:off + w1] = m1[:, :, s:s + cnt].reshape(NCORES, 128, w1)
        md[:, :, off + w1:off + w1 + w2] = m2[:, :, s:s + cnt].reshape(
            NCORES, 128, w2)
        s += cnt
        off += w1 + w2
    md = md.view(ml_dtypes.float8_e4m3)

    tft = np.ascontiguousarray(np.asarray(track_features, dtype=np.float32).T)
    cft = np.ascontiguousarray(np.asarray(current_features, dtype=np.float32).T)
    tb = np.ascontiguousarray(np.asarray(track_boxes, dtype=np.float32))
    cbt = np.ascontiguousarray(np.asarray(current_boxes, dtype=np.float32).T)

    in_maps = [
        {"md": md[c], "tft": tft, "cft": cft, "tb": tb, "cbt": cbt}
        for c in range(NCORES)
    ]
    nc = _build()
    res = run_bass_kernel_spmd(nc, in_maps, core_ids=list(range(NCORES)),
                               trace=_CACHE.get("trace", False))
    _CACHE["last_exec_time_ns"] = res.exec_time_ns
    out = np.empty((N1, N2), dtype=np.float32)
    for c in range(NCORES):
        out[:, c * NS:(c + 1) * NS] = np.asarray(
            res.results[c]["out"])[:, c * NS:(c + 1) * NS]
    return out
